# revision 1
# baseline (speedup 1.0000x reference)
# Trainium2 Bass kernel for nn_DetectionLoss (B=32, N=25200, M=200, C=80).
#
# Strategy: pure data-parallel over batch (4 batches per core, 8 cores).
# The reference only reads pred_bbox[:, :M] and pred_cls[:, :M], so only
# those slices are shipped to the device. Each core computes per-partition
# partial sums of the four loss terms; the host does the final (tiny)
# cross-core reduction and mean/lambda arithmetic in float64.
#
# Device inputs per core (host-packed into device layout):
#   boxes [100, 64] f32:    pred|gt boxes, [p, s, j=(b,k), c] packed
#   clsmask [100, 1280] bf16: cols 0:640 cls logits, 640:1280 one-hot mask
#   obj   [120, 900] bf16:  rows 0:112 all 4*25200 obj logits (flat reshape),
#                           rows 112:116 -x of positives (softplus(-x) term),
#                           rows 116:120 +x of positives (correction term),
#                           positives rows padded with -30 (softplus == 0)
# DMAs are chunked across the three DMA queues so ACT compute overlaps the
# transfers. Output per core: partials [128, 8] f32:
#   col 0 sum(iou), col 1 sum((enclose-union)/(enclose+eps)),
#   col 2 softplus sums (split by partition range as above),
#   col 3 sum(logsumexp), col 4 sum(picked logit)

import numpy as np

B, N, M, C = 32, 25200, 200, 80
NCORES = 8
BPC = B // NCORES          # 4 batches per core
KP = 2                     # anchors per (partition, batch)
P_PAIRS = M // KP          # 100 partitions for pair-space tiles
NPAIR = BPC * KP           # 8 pairs per partition
P_OBJ, F_OBJ = 112, 900    # 4*25200 = 112*900
EPS = 1e-7
PAD = -30.0                # softplus(PAD) == 0 exactly in f32
W_CM = 2 * NPAIR * C       # 1280

_CACHED_NC = None


def _emit(nc, tc, mybir, boxes, clsmask, obj, out):
    f32 = mybir.dt.float32
    bf16 = mybir.dt.bfloat16
    Alu = mybir.AluOpType
    Act = mybir.ActivationFunctionType

    with tc.tile_pool(name="main", bufs=1) as pool:
        ACC = pool.tile([128, 8], f32, name="ACC")
        nc.vector.memset(ACC[:], 0.0)

        BX = pool.tile([P_PAIRS, 64], f32, name="BX")
        CM = pool.tile([P_PAIRS, W_CM], bf16, name="CM")
        OBJ = pool.tile([120, F_OBJ], bf16, name="OBJ")
        # Chunked DMAs across the three DMA-capable queues; small boxes first
        # so the DVE chain starts early, obj/cls chunks pipeline with ACT.
        nc.sync.dma_start(out=BX[:], in_=boxes.ap())
        nc.scalar.dma_start(out=OBJ[0:64], in_=obj.ap()[0:64])
        nc.sync.dma_start(out=OBJ[64:120], in_=obj.ap()[64:120])
        nc.gpsimd.dma_start(out=CM[0:64], in_=clsmask.ap()[0:64])
        nc.gpsimd.dma_start(out=CM[64:100], in_=clsmask.ap()[64:100])

        # ---------------- objectness softplus (two row-chunks) ----------------
        Eo = pool.tile([120, F_OBJ], f32, name="Eo")
        for r0, r1 in ((0, 64), (64, 120)):
            nc.scalar.activation(Eo[r0:r1], OBJ[r0:r1], Act.Exp)
        # ---------------- classification: exp of logits (two row-chunks) ------
        Ec = pool.tile([P_PAIRS, NPAIR, C], f32, name="Ec")
        sums = pool.tile([P_PAIRS, NPAIR], f32, name="sums")
        lse = pool.tile([P_PAIRS, NPAIR], f32, name="lse")
        prod = pool.tile([P_PAIRS, NPAIR * C], f32, name="prod")
        CLf = CM[:, 0:NPAIR * C]
        MKf = CM[:, NPAIR * C:W_CM]
        for r0, r1 in ((0, 64), (64, 100)):
            nc.scalar.activation(
                Ec[r0:r1].rearrange("p a c -> p (a c)"), CLf[r0:r1], Act.Exp
            )
        # softplus(a)+softplus(b) = log((1+e^a)(1+e^b)): build the 4-way
        # product tree on GpSimd (idle after DMA issue) so the Ln pass shrinks
        # from [120,900] to [120,225]. Per-partition sums are preserved.
        Vv = pool.tile([120, F_OBJ], f32, name="Vv")
        M1 = pool.tile([120, F_OBJ // 2], f32, name="M1")
        M2 = pool.tile([120, F_OBJ // 4], f32, name="M2")
        Lg = pool.tile([120, F_OBJ // 4], f32, name="Lg")
        nc.vector.tensor_scalar_add(Vv[:], Eo[:], 1.0)
        nc.vector.tensor_mul(M1[:], Vv[:, 0:450], Vv[:, 450:900])
        nc.vector.tensor_mul(M2[:], M1[:, 0:225], M1[:, 225:450])
        nc.scalar.activation(Lg[:], M2[:], Act.Ln, accum_out=ACC[0:120, 2:3])
        for r0, r1 in ((0, 64), (64, 100)):
            nc.vector.reduce_sum(out=sums[r0:r1], in_=Ec[r0:r1],
                                 axis=mybir.AxisListType.X)
            nc.vector.scalar_tensor_tensor(
                prod[r0:r1], CLf[r0:r1], 1.0, MKf[r0:r1], Alu.mult, Alu.mult,
                accum_out=ACC[r0:r1, 4:5],
            )
        nc.scalar.activation(lse[:], sums[:], Act.Ln,
                             accum_out=ACC[0:P_PAIRS, 3:4])

        # ---------------- bbox GIoU term ----------------
        PB = BX[:].rearrange("p (s j c) -> p s j c", s=2, c=4)
        cxcy = PB[:, :, :, 0:2]
        wh = PB[:, :, :, 2:4]
        C1 = pool.tile([P_PAIRS, 2, NPAIR, 2], f32, name="C1")
        C2 = pool.tile([P_PAIRS, 2, NPAIR, 2], f32, name="C2")
        nc.vector.scalar_tensor_tensor(C1[:], wh, -0.5, cxcy, Alu.mult, Alu.add)
        nc.vector.scalar_tensor_tensor(C2[:], wh, 0.5, cxcy, Alu.mult, Alu.add)
        I1 = pool.tile([P_PAIRS, NPAIR, 2], f32, name="I1")
        I2 = pool.tile([P_PAIRS, NPAIR, 2], f32, name="I2")
        E1 = pool.tile([P_PAIRS, NPAIR, 2], f32, name="E1")
        E2 = pool.tile([P_PAIRS, NPAIR, 2], f32, name="E2")
        nc.vector.tensor_tensor(I1[:], C1[:, 0], C1[:, 1], Alu.max)
        nc.vector.tensor_tensor(I2[:], C2[:, 0], C2[:, 1], Alu.min)
        nc.vector.tensor_tensor(E1[:], C1[:, 0], C1[:, 1], Alu.min)
        nc.vector.tensor_tensor(E2[:], C2[:, 0], C2[:, 1], Alu.max)
        ID = pool.tile([P_PAIRS, NPAIR, 2], f32, name="ID")
        IDr = pool.tile([P_PAIRS, NPAIR, 2], f32, name="IDr")
        ED = pool.tile([P_PAIRS, NPAIR, 2], f32, name="ED")
        nc.vector.tensor_sub(ID[:], I2[:], I1[:])
        nc.vector.tensor_relu(IDr[:], ID[:])
        nc.vector.tensor_sub(ED[:], E2[:], E1[:])
        inter = pool.tile([P_PAIRS, NPAIR], f32, name="inter")
        encl = pool.tile([P_PAIRS, NPAIR], f32, name="encl")
        nc.vector.tensor_mul(inter[:], IDr[:, :, 0], IDr[:, :, 1])
        nc.vector.tensor_mul(encl[:], ED[:, :, 0], ED[:, :, 1])
        A = pool.tile([P_PAIRS, 2, NPAIR], f32, name="A")
        nc.vector.tensor_mul(A[:], PB[:, :, :, 2], PB[:, :, :, 3])
        asum = pool.tile([P_PAIRS, NPAIR], f32, name="asum")
        nc.vector.tensor_add(asum[:], A[:, 0], A[:, 1])
        U = pool.tile([P_PAIRS, NPAIR], f32, name="U")
        nc.vector.scalar_tensor_tensor(U[:], inter[:], -1.0, asum[:],
                                       Alu.mult, Alu.add)
        Ue = pool.tile([P_PAIRS, NPAIR], f32, name="Ue")
        Ur = pool.tile([P_PAIRS, NPAIR], f32, name="Ur")
        nc.vector.tensor_scalar_add(Ue[:], U[:], EPS)
        nc.vector.reciprocal(Ur[:], Ue[:])
        # NOTE: tensor_tensor_reduce wedges the device (NRT_EXEC_UNIT_UNRECOVERABLE)
        # on this runtime; scalar_tensor_tensor's accum_out path works.
        t8a = pool.tile([P_PAIRS, NPAIR], f32, name="t8a")
        nc.vector.scalar_tensor_tensor(
            t8a[:], inter[:], 1.0, Ur[:], Alu.mult, Alu.mult,
            accum_out=ACC[0:P_PAIRS, 0:1],
        )
        EmU = pool.tile([P_PAIRS, NPAIR], f32, name="EmU")
        Ee = pool.tile([P_PAIRS, NPAIR], f32, name="Ee")
        Er = pool.tile([P_PAIRS, NPAIR], f32, name="Er")
        nc.vector.tensor_sub(EmU[:], encl[:], U[:])
        nc.vector.tensor_scalar_add(Ee[:], encl[:], EPS)
        nc.vector.reciprocal(Er[:], Ee[:])
        t8b = pool.tile([P_PAIRS, NPAIR], f32, name="t8b")
        nc.vector.scalar_tensor_tensor(
            t8b[:], EmU[:], 1.0, Er[:], Alu.mult, Alu.mult,
            accum_out=ACC[0:P_PAIRS, 1:2],
        )

        nc.sync.dma_start(out=out.ap(), in_=ACC[:])


def build_bass():
    global _CACHED_NC
    if _CACHED_NC is not None:
        return _CACHED_NC
    import concourse.bacc as bacc
    import concourse.tile as tile
    import concourse.mybir as mybir

    f32 = mybir.dt.float32
    bf16 = mybir.dt.bfloat16
    Act = mybir.ActivationFunctionType

    class FastTileContext(tile.TileContext):
        # Same as TileContext._drain_and_barrier but: sem-only barrier and no
        # trailing second barrier — trims the kernel-tail cost.
        def _drain_and_barrier(self, tick_clock, wait_clock):
            drain_inst = self.nc.sync.drain()
            wait_clock.add_sem_waits(
                drain_inst.ins, tile.ScopedClock({None: tick_clock.global_clock})
            )
            self.nc.all_engine_barrier(sem_only=True)
            popped = self.nc._tile_sem_poison_stack.pop()
            assert popped is self._sem_poison
            self.nc.clear_and_free_semaphores(list(self.sems.allocated().values()))

    nc = bacc.Bacc("TRN2", target_bir_lowering=False, debug=False,
                   num_devices=NCORES)
    boxes = nc.dram_tensor("boxes", [P_PAIRS, 64], f32, kind="ExternalInput")
    clsmask = nc.dram_tensor("clsmask", [P_PAIRS, W_CM], bf16,
                             kind="ExternalInput")
    obj = nc.dram_tensor("obj", [120, F_OBJ], bf16, kind="ExternalInput")
    out = nc.dram_tensor("partials", [128, 8], f32, kind="ExternalOutput")
    with FastTileContext(nc) as tc:
        _emit(nc, tc, mybir, boxes, clsmask, obj, out)

    # Route every Exp/Ln to the one table that holds both, so the kernel pays
    # a single ACT_TABLE_LOAD instead of ping-ponging between per-func tables.
    # Patch is scoped to this compile; table ids are positional so only the
    # membership sets are altered (ids stay valid).
    orig_tables = bacc.get_activation_tables

    def _merged_tables(arch):
        out_d = {}
        for name, s in orig_tables(arch).items():
            s2 = set(s)
            if name != "natural_log_exp_and_others":
                s2.discard(Act.Exp)
                s2.discard(Act.Ln)
            out_d[name] = s2
        return out_d

    bacc.get_activation_tables = _merged_tables
    try:
        nc.compile()
    finally:
        bacc.get_activation_tables = orig_tables

    # Drop a spurious default-table InstLoadActFuncSet: when two loads appear
    # with no activation between them, the first is dead and its 1.3us sits
    # right before the first Exp on the critical path.
    for blk in nc.main_func.blocks:
        loads = []
        acts_seen = set()
        for idx, ins in enumerate(blk.instructions):
            tn = type(ins).__name__
            if tn == "InstLoadActFuncSet":
                loads.append((idx, ins))
            elif tn == "InstActivation":
                acts_seen.add(len(loads))
        if len(loads) == 2 and 1 not in acts_seen and loads[0][1].sync_info is None:
            blk.instructions.pop(loads[0][0])

    _CACHED_NC = nc
    return nc


def make_in_maps(pred_bbox, pred_obj, pred_cls, gt_boxes, gt_labels):
    import ml_dtypes

    bf16 = ml_dtypes.bfloat16
    labels = np.asarray(gt_labels).astype(np.int64)
    cls_ar = np.arange(C)
    in_maps = []
    for core in range(NCORES):
        bs = slice(core * BPC, (core + 1) * BPC)

        boxes = np.empty((P_PAIRS, 64), np.float32)
        pb = np.asarray(pred_bbox[bs, :M], np.float32).reshape(BPC, P_PAIRS, KP, 4)
        gb = np.asarray(gt_boxes[bs], np.float32).reshape(BPC, P_PAIRS, KP, 4)
        boxes[:, 0:32] = pb.transpose(1, 0, 2, 3).reshape(P_PAIRS, 32)
        boxes[:, 32:64] = gb.transpose(1, 0, 2, 3).reshape(P_PAIRS, 32)

        clsmask = np.empty((P_PAIRS, W_CM), bf16)
        cl = np.asarray(pred_cls[bs, :M], np.float32).reshape(BPC, P_PAIRS, KP, C)
        clsmask[:, 0:NPAIR * C] = cl.transpose(1, 0, 2, 3).reshape(
            P_PAIRS, NPAIR * C
        ).astype(bf16)
        lab = labels[bs].reshape(BPC, P_PAIRS, KP)
        onehot = (lab[..., None] == cls_ar).astype(np.float32)
        clsmask[:, NPAIR * C:W_CM] = onehot.transpose(1, 0, 2, 3).reshape(
            P_PAIRS, NPAIR * C
        ).astype(bf16)

        po = np.asarray(pred_obj[bs], np.float32)
        obj = np.full((120, F_OBJ), PAD, np.float32)
        obj[0:P_OBJ] = po.reshape(P_OBJ, F_OBJ)
        obj[P_OBJ:P_OBJ + BPC, 0:M] = -po[:, :M]
        obj[P_OBJ + BPC:P_OBJ + 2 * BPC, 0:M] = po[:, :M]

        in_maps.append({"boxes": boxes, "clsmask": clsmask,
                        "obj": obj.astype(bf16)})
    return in_maps


def finalize(per_core_partials):
    s_iou = s_ratio = s_all = s_pos = s_posplus = s_lse = s_picked = 0.0
    for p in per_core_partials:
        p = p.astype(np.float64)
        s_iou += p[:, 0].sum()
        s_ratio += p[:, 1].sum()
        s_all += p[0:P_OBJ, 2].sum()
        s_pos += p[P_OBJ:P_OBJ + BPC, 2].sum()
        s_posplus += p[P_OBJ + BPC:P_OBJ + 2 * BPC, 2].sum()
        s_lse += p[:, 3].sum()
        s_picked += p[:, 4].sum()
    n_pos = B * M
    n_neg = B * (N - M)
    loss_bbox = 5.0 * (n_pos - s_iou + s_ratio) / n_pos
    loss_obj = s_pos / n_pos + 0.5 * (s_all - s_posplus) / n_neg
    loss_cls = (s_lse - s_picked) / n_pos
    total = loss_bbox + loss_obj + loss_cls
    return np.array([total, loss_bbox, loss_obj, loss_cls], dtype=np.float32)


def kernel(pred_bbox, pred_obj, pred_cls, gt_boxes, gt_labels):
    from concourse.bass_utils import run_bass_kernel_spmd

    nc = build_bass()
    in_maps = make_in_maps(pred_bbox, pred_obj, pred_cls, gt_boxes, gt_labels)
    res = run_bass_kernel_spmd(nc, in_maps, core_ids=list(range(NCORES)))
    return finalize([r["partials"] for r in res.results])

